# revision 4
# baseline (speedup 1.0000x reference)
"""CenterLoss kernel for 8 Trainium2 NeuronCores (Bass/Tile).

Problem (hardcoded, from nn_CenterLoss):
    h      [262144, 256] f32
    d      [262144]      int   (class ids in [0, 100000))
    center [100000, 256] f32
    returns (loss, new_center):
        loss       = mean((h - center[d])**2)                       scalar f32
        new_center = center + 0.1 * where(count>0, segmean(h)-center, 0)

Sharding strategy: shard along n_class. Core c owns classes
[c*12500, (c+1)*12500) and the matching center rows; the host routes each
batch row to the core owning its class (sorted by class id). Segment sums
then never need a cross-device reduction.

Device algorithm per core:
  - Classes are processed in 98 tiles of 128. For class tile j the batch
    rows belonging to it (padded with zero rows to n_j*128, n_j static and
    identical on every core so the SPMD program is uniform) are multiplied
    by an on-device-built one-hot selector and accumulated into PSUM:
        sums[128, 256] += onehot[128b, 128c].T @ h[128b, 256]
  - loss uses the expansion sum|h|^2 - 2*sum_k s_k.c_k + sum_k n_k|c_k|^2
    so center[d] is never gathered.
  - new_center tile = c + (alpha*present) * (sums*inv_count - c)
"""

import numpy as np

BATCH = 262144
F = 256
C = 100000
ALPHA = 0.1
N_CORES = 8
P = 128
C_SHARD = C // N_CORES          # 12500 classes per core
CT = (C_SHARD + P - 1) // P     # 98 class tiles per core
C_PAD = CT * P                  # 12544 rows incl. padding

_prog_cache = {}


def _shard_inputs(h, d, center):
    """Route batch rows to the core owning their class; build per-core,
    per-class-tile padded blocks laid out for single-descriptor DMAs.

    Returns (in_maps, n_j) where n_j[j] is the static number of 128-row
    batch sub-tiles feeding class tile j (same for every core)."""
    h = np.ascontiguousarray(np.asarray(h, dtype=np.float32))
    d = np.asarray(d).astype(np.int64)
    center = np.asarray(center, dtype=np.float32)

    order = np.argsort(d, kind="stable")
    ds = d[order]
    core_bounds = np.searchsorted(ds, np.arange(N_CORES + 1) * C_SHARD)
    counts_global = np.bincount(d, minlength=C)

    tile_rows_all = np.zeros((N_CORES, CT), dtype=np.int64)
    cnt_pads = []
    for c in range(N_CORES):
        cnt_pad = np.zeros(C_PAD, np.int64)
        cnt_pad[:C_SHARD] = counts_global[c * C_SHARD:(c + 1) * C_SHARD]
        cnt_pads.append(cnt_pad)
        tile_rows_all[c] = cnt_pad.reshape(CT, P).sum(1)

    n_j = np.maximum(np.ceil(tile_rows_all / P).astype(np.int64).max(0), 1)
    nbt = int(n_j.sum())

    in_maps = []
    for c in range(N_CORES):
        lo, hi = core_bounds[c], core_bounds[c + 1]
        rows_idx = order[lo:hi]
        dl = (ds[lo:hi] - c * C_SHARD).astype(np.float32)
        tile_rows = tile_rows_all[c]
        tb = np.concatenate([[0], np.cumsum(tile_rows)])

        h_blk = np.zeros((P, nbt, F), np.float32)
        d_blk = np.zeros((P, nbt), np.float32)
        col = 0
        hs = h[rows_idx]  # this core's rows, class-sorted
        for j in range(CT):
            r0, r1 = tb[j], tb[j + 1]
            nr = r1 - r0
            njj = int(n_j[j])
            bh = np.zeros((njj * P, F), np.float32)
            bd = np.full((njj * P,), j * P, np.float32)
            bh[:nr] = hs[r0:r1]
            bd[:nr] = dl[r0:r1]
            h_blk[:, col:col + njj, :] = bh.reshape(njj, P, F).transpose(1, 0, 2)
            d_blk[:, col:col + njj] = bd.reshape(njj, P).T
            col += njj

        cshard = np.zeros((C_PAD, F), np.float32)
        cshard[:C_SHARD] = center[c * C_SHARD:(c + 1) * C_SHARD]

        in_maps.append({
            "h_blk": np.ascontiguousarray(h_blk.reshape(P, nbt * F)),
            "d_blk": np.ascontiguousarray(d_blk),
            "counts": cnt_pads[c].astype(np.float32),
            "center_in": cshard,
        })
    return in_maps, tuple(int(x) for x in n_j)


def _emit_body(nc, tc, mybir, pools, n_j, aps):
    """One full pass: segment sums + center update + loss partials."""
    import concourse.bass as bass  # noqa: F401

    dtf = mybir.dt.float32
    alu = mybir.AluOpType
    h_blk, d_blk, counts, center_in, center_out, lossp = aps
    cpool, hpool, dpool, ohpool, clspool, accpool, pspool, pslpool = pools

    iota_i = cpool.tile([P, P], mybir.dt.int32, tag="iota_i")
    nc.gpsimd.iota(iota_i[:], pattern=[[1, P]], base=0, channel_multiplier=0)
    iota_f = cpool.tile([P, P], dtf, tag="iota_f")
    nc.vector.tensor_copy(iota_f[:], iota_i[:])
    ones = cpool.tile([P, 1], dtf, tag="ones")
    nc.vector.memset(ones[:], 1.0)

    nbt = int(sum(n_j))
    # per-partition loss accumulators: col 0 = sum h^2, col 1 = sum s.c
    acc = accpool.tile([P, 4], dtf, tag="acc")
    nc.vector.memset(acc[:], 0.0)
    # staging columns (reduced once at the end): per-subtile sum h^2,
    # per-class-tile sum s.c and |c|^2
    hsq_stage = accpool.tile([P, nbt], dtf, tag="hsq_stage")
    cross_stage = accpool.tile([P, CT], dtf, tag="cross_stage")
    csq_all = accpool.tile([P, CT], dtf, tag="csq_all")

    # counts for all class tiles: [P, CT]; partition = class within tile
    cnt_all = accpool.tile([P, CT], dtf, tag="cnt_all")
    nc.sync.dma_start(cnt_all[:], counts[:].rearrange("(t p) -> p t", p=P))
    inv_all = accpool.tile([P, CT], dtf, tag="inv_all")
    safe_all = accpool.tile([P, CT], dtf, tag="safe_all")
    nc.vector.tensor_scalar_max(safe_all[:], cnt_all[:], 1.0)
    nc.vector.reciprocal(inv_all[:], safe_all[:])
    # alpha * (count > 0)
    pres_all = accpool.tile([P, CT], dtf, tag="pres_all")
    nc.vector.tensor_scalar(
        out=pres_all[:], in0=cnt_all[:], scalar1=0.5, scalar2=ALPHA,
        op0=alu.is_ge, op1=alu.mult,
    )

    col = 0
    for j in range(CT):
        nj = n_j[j]
        ht = hpool.tile([P, nj * F], dtf, tag="ht")
        nc.sync.dma_start(ht[:], h_blk[:, col * F:(col + nj) * F])
        dt_t = dpool.tile([P, nj], dtf, tag="dt")
        nc.sync.dma_start(dt_t[:], d_blk[:, col:col + nj])

        ps = pspool.tile([P, F], dtf, tag="ps")
        for s in range(nj):
            oh = ohpool.tile([P, P], dtf, tag="oh")
            # onehot[b, c] = 1 iff (iota_c - dloc_b) == -128j
            nc.vector.tensor_scalar(
                out=oh[:], in0=iota_f[:], scalar1=dt_t[:, s:s + 1],
                scalar2=float(-(j * P)), op0=alu.subtract, op1=alu.is_equal,
            )
            nc.tensor.matmul(
                ps[:], lhsT=oh[:], rhs=ht[:, s * F:(s + 1) * F],
                start=(s == 0), stop=(s == nj - 1),
            )
            # sum_f h^2 for this sub-tile (ACT square, fused accumulate)
            hsq = ohpool.tile([P, F], dtf, tag="hsq")
            nc.scalar.activation(
                hsq[:], ht[:, s * F:(s + 1) * F],
                mybir.ActivationFunctionType.Square,
                accum_out=hsq_stage[:, col + s:col + s + 1],
            )
        col += nj

        ct_t = clspool.tile([P, F], dtf, tag="ct")
        nc.sync.dma_start(ct_t[:], center_in[j * P:(j + 1) * P, :])

        # mean = sums * (1/max(n,1));  new_c = c + alpha*present*(mean - c)
        mean = clspool.tile([P, F], dtf, tag="mean")
        nc.vector.tensor_scalar_mul(mean[:], ps[:], inv_all[:, j:j + 1])
        dcm = clspool.tile([P, F], dtf, tag="dcm")
        nc.vector.tensor_tensor(out=dcm[:], in0=mean[:], in1=ct_t[:],
                                op=alu.subtract)
        newc = clspool.tile([P, F], dtf, tag="newc")
        nc.vector.tensor_scalar(
            out=newc[:], in0=dcm[:], scalar1=pres_all[:, j:j + 1],
            scalar2=None, op0=alu.mult,
        )
        nc.vector.tensor_add(newc[:], newc[:], ct_t[:])
        nc.sync.dma_start(center_out[j * P:(j + 1) * P, :], newc[:])

        # loss partials: cross_stage[:,j] = sum_f s*c ; csq_all[:,j] = sum_f c^2
        sc = clspool.tile([P, F], dtf, tag="sc")
        nc.vector.tensor_mul(sc[:], ps[:], ct_t[:])
        nc.vector.tensor_reduce(cross_stage[:, j:j + 1], sc[:],
                                axis=mybir.AxisListType.X, op=alu.add)
        csq = clspool.tile([P, F], dtf, tag="csq")
        nc.vector.tensor_mul(csq[:], ct_t[:], ct_t[:])
        nc.vector.tensor_reduce(csq_all[:, j:j + 1], csq[:],
                                axis=mybir.AxisListType.X, op=alu.add)

    # fold staging columns: acc[:,0]=sum h^2, acc[:,1]=sum s.c,
    # acc[:,2]=sum_j count_j*|c_j|^2
    nc.vector.tensor_reduce(acc[:, 0:1], hsq_stage[:],
                            axis=mybir.AxisListType.X, op=alu.add)
    nc.vector.tensor_reduce(acc[:, 1:2], cross_stage[:],
                            axis=mybir.AxisListType.X, op=alu.add)
    wc_all = accpool.tile([P, CT], dtf, tag="wc_all")
    nc.vector.tensor_tensor(out=wc_all[:], in0=csq_all[:], in1=cnt_all[:],
                            op=alu.mult)
    nc.vector.tensor_reduce(acc[:, 2:3], wc_all[:], axis=mybir.AxisListType.X,
                            op=alu.add)

    # reduce partition dim: lossp[1,4] = ones.T @ acc
    psl = pslpool.tile([1, 4], dtf, tag="psl")
    nc.tensor.matmul(psl[:], lhsT=ones[:], rhs=acc[:], start=True, stop=True)
    lt = accpool.tile([1, 4], dtf, tag="lt")
    nc.vector.tensor_copy(lt[:], psl[:])
    nc.sync.dma_start(lossp[:], lt[:])


def build_program(n_j, reps=1):
    """Compile the SPMD program for the given static per-class-tile batch
    sub-tile counts. reps>1 wraps the body in a hardware loop (timing)."""
    key = (tuple(n_j), reps)
    if key in _prog_cache:
        return _prog_cache[key]
    import concourse.bacc as bacc
    import concourse.mybir as mybir
    import concourse.tile as tile

    nbt = int(sum(n_j))
    dtf = mybir.dt.float32
    nc = bacc.Bacc("TRN2", target_bir_lowering=False, debug=False,
                   num_devices=N_CORES)
    h_blk = nc.dram_tensor("h_blk", [P, nbt * F], dtf, kind="ExternalInput").ap()
    d_blk = nc.dram_tensor("d_blk", [P, nbt], dtf, kind="ExternalInput").ap()
    counts = nc.dram_tensor("counts", [C_PAD], dtf, kind="ExternalInput").ap()
    center_in = nc.dram_tensor("center_in", [C_PAD, F], dtf,
                               kind="ExternalInput").ap()
    center_out = nc.dram_tensor("center_out", [C_PAD, F], dtf,
                                kind="ExternalOutput").ap()
    lossp = nc.dram_tensor("lossp", [1, 4], dtf, kind="ExternalOutput").ap()
    aps = (h_blk, d_blk, counts, center_in, center_out, lossp)

    with tile.TileContext(nc) as tc:
        with (
            tc.tile_pool(name="const", bufs=1) as cpool,
            tc.tile_pool(name="hp", bufs=4) as hpool,
            tc.tile_pool(name="dp", bufs=4) as dpool,
            tc.tile_pool(name="ohp", bufs=4) as ohpool,
            tc.tile_pool(name="cls", bufs=3) as clspool,
            tc.tile_pool(name="accp", bufs=1) as accpool,
            tc.tile_pool(name="ps", bufs=4, space="PSUM") as pspool,
            tc.tile_pool(name="psl", bufs=1, space="PSUM") as pslpool,
        ):
            pools = (cpool, hpool, dpool, ohpool, clspool, accpool,
                     pspool, pslpool)
            if reps == 1:
                _emit_body(nc, tc, mybir, pools, n_j, aps)
            else:
                with tc.For_i(0, reps, 1):
                    _emit_body(nc, tc, mybir, pools, n_j, aps)
    nc.compile()
    _prog_cache[key] = nc
    return nc


def _unshard(results):
    new_center = np.concatenate(
        [results[c]["center_out"][:C_SHARD] for c in range(N_CORES)], axis=0)
    tot = np.zeros(3, np.float64)
    for c in range(N_CORES):
        lp = results[c]["lossp"][0]
        tot += lp[:3].astype(np.float64)
    loss = (tot[0] - 2.0 * tot[1] + tot[2]) / (BATCH * F)
    return np.float32(loss), new_center


def kernel(h, d, center):
    from concourse.bass_utils import run_bass_kernel_spmd

    in_maps, n_j = _shard_inputs(h, d, center)
    nc = build_program(n_j, reps=1)
    res = run_bass_kernel_spmd(nc, in_maps, core_ids=list(range(N_CORES)))
    return _unshard(res.results)


# revision 7
# speedup vs baseline: 1.8149x; 1.8149x over previous
"""CenterLoss kernel for 8 Trainium2 NeuronCores (Bass/Tile).

Problem (hardcoded, from nn_CenterLoss):
    h      [262144, 256] f32
    d      [262144]      int   (class ids in [0, 100000))
    center [100000, 256] f32
    returns (loss, new_center):
        loss       = mean((h - center[d])**2)                       scalar f32
        new_center = center + 0.1 * where(count>0, segmean(h)-center, 0)

Sharding strategy: shard along n_class. Core c owns classes
[c*12500, (c+1)*12500) and the matching center rows; the host routes each
batch row to the core owning its class (sorted by class id). Segment sums
then never need a cross-device reduction.

Device algorithm per core:
  - Classes are processed in 98 tiles of 128. For class tile j the batch
    rows belonging to it (padded with zero rows to n_j*128, n_j static and
    identical on every core so the SPMD program is uniform) are multiplied
    by an on-device-built one-hot selector and accumulated into PSUM:
        sums[128, 256] += onehot[128b, 128c].T @ h[128b, 256]
  - loss uses the expansion sum|h|^2 - 2*sum_k s_k.c_k + sum_k n_k|c_k|^2
    so center[d] is never gathered.
  - new_center tile = a_inv*sums + oma*c  with a_inv = alpha*present/count,
    oma = 1 - alpha*present (both precomputed per class on device).
  - All DRAM streams are laid out partition-major on the host so each
    multi-class-tile group moves with a single fully-contiguous DMA.
"""

import numpy as np

BATCH = 262144
F = 256
C = 100000
ALPHA = 0.1
N_CORES = 8
P = 128
C_SHARD = C // N_CORES          # 12500 classes per core
CT = (C_SHARD + P - 1) // P     # 98 class tiles per core
C_PAD = CT * P                  # 12544 rows incl. padding

H_GROUP_COLS = 12               # max batch sub-tiles per h DMA group
C_GROUP = 7                     # class tiles per center DMA group (98 = 14*7)
ACT_HSQ_MOD = 2                 # sub-tile idx % MOD == 0 -> |h|^2 on ACT

_prog_cache = {}


def _h_groups(n_j):
    """Greedily group consecutive class tiles so each h DMA moves at most
    H_GROUP_COLS 128-row sub-tiles."""
    groups = []
    cur = []
    cols = 0
    for j, nj in enumerate(n_j):
        if cur and cols + nj > H_GROUP_COLS:
            groups.append(cur)
            cur, cols = [], 0
        cur.append(j)
        cols += nj
    if cur:
        groups.append(cur)
    return groups


def _shard_inputs(h, d, center):
    """Route batch rows to the core owning their class; build per-core
    partition-major blocks so every device DMA is contiguous.

    Returns (in_maps, n_j)."""
    h = np.ascontiguousarray(np.asarray(h, dtype=np.float32))
    d = np.asarray(d).astype(np.int64)
    center = np.asarray(center, dtype=np.float32)

    order = np.argsort(d, kind="stable")
    ds = d[order]
    core_bounds = np.searchsorted(ds, np.arange(N_CORES + 1) * C_SHARD)
    counts_global = np.bincount(d, minlength=C)

    tile_rows_all = np.zeros((N_CORES, CT), dtype=np.int64)
    cnt_pads = []
    for c in range(N_CORES):
        cnt_pad = np.zeros(C_PAD, np.int64)
        cnt_pad[:C_SHARD] = counts_global[c * C_SHARD:(c + 1) * C_SHARD]
        cnt_pads.append(cnt_pad)
        tile_rows_all[c] = cnt_pad.reshape(CT, P).sum(1)

    n_j = np.maximum(np.ceil(tile_rows_all / P).astype(np.int64).max(0), 1)
    nbt = int(n_j.sum())
    h_groups = _h_groups(n_j)

    in_maps = []
    for c in range(N_CORES):
        lo, hi = core_bounds[c], core_bounds[c + 1]
        rows_idx = order[lo:hi]
        dl = (ds[lo:hi] - c * C_SHARD).astype(np.float32)
        tile_rows = tile_rows_all[c]
        tb = np.concatenate([[0], np.cumsum(tile_rows)])
        hs = h[rows_idx]  # this core's rows, class-sorted

        # per-class-tile pieces, partition-major [P, n_j, F] / [P, n_j]
        pieces_h, pieces_d = [], []
        for j in range(CT):
            r0, r1 = tb[j], tb[j + 1]
            nr = r1 - r0
            njj = int(n_j[j])
            bh = np.zeros((njj * P, F), np.float32)
            bd = np.full((njj * P,), j * P, np.float32)
            bh[:nr] = hs[r0:r1]
            bd[:nr] = dl[r0:r1]
            pieces_h.append(bh.reshape(njj, P, F).transpose(1, 0, 2))
            pieces_d.append(bd.reshape(njj, P).T)

        # h: concatenated per h-group, each group contiguous [P, cols*F]
        h_parts = []
        for g in h_groups:
            grp = np.concatenate([pieces_h[j] for j in g], axis=1)  # [P,cols,F]
            h_parts.append(grp.reshape(P, -1))
        h_blk = np.concatenate([p.reshape(-1) for p in h_parts])

        d_all = np.concatenate(pieces_d, axis=1)  # [P, nbt]

        counts_pm = cnt_pads[c].astype(np.float32).reshape(CT, P).T  # [P, CT]

        cshard = np.zeros((C_PAD, F), np.float32)
        cshard[:C_SHARD] = center[c * C_SHARD:(c + 1) * C_SHARD]
        # partition-major grouped center: [P, CT*F], tile t cols [t*F,(t+1)*F]
        center_pm = cshard.reshape(CT, P, F).transpose(1, 0, 2).reshape(P, CT * F)

        in_maps.append({
            "h_blk": np.ascontiguousarray(h_blk),
            "d_all": np.ascontiguousarray(d_all),
            "counts": np.ascontiguousarray(counts_pm),
            "center_in": np.ascontiguousarray(center_pm),
        })
    return in_maps, tuple(int(x) for x in n_j)


def _emit_body(nc, tc, mybir, pools, n_j, aps):
    """One full pass: segment sums + center update + loss partials."""
    dtf = mybir.dt.float32
    alu = mybir.AluOpType
    h_blk, d_all_d, counts, center_in, center_out, lossp = aps
    cpool, hpool, ohpool, clspool, accpool, pspool, pslpool = pools

    nbt = int(sum(n_j))
    h_groups = _h_groups(n_j)
    col_of = np.concatenate([[0], np.cumsum(n_j)]).astype(int)

    iota_i = cpool.tile([P, P], mybir.dt.int32, tag="iota_i")
    nc.gpsimd.iota(iota_i[:], pattern=[[1, P]], base=0, channel_multiplier=0)
    iota_f = cpool.tile([P, P], dtf, tag="iota_f")
    nc.vector.tensor_copy(iota_f[:], iota_i[:])
    ones = cpool.tile([P, 1], dtf, tag="ones")
    nc.vector.memset(ones[:], 1.0)

    # resident tables: sorted local class ids + per-class factors
    d_all = accpool.tile([P, nbt], dtf, tag="d_all")
    nc.sync.dma_start(d_all[:], d_all_d[:])
    cnt_all = accpool.tile([P, CT], dtf, tag="cnt_all")
    nc.sync.dma_start(cnt_all[:], counts[:])
    safe_all = accpool.tile([P, CT], dtf, tag="safe_all")
    nc.vector.tensor_scalar_max(safe_all[:], cnt_all[:], 1.0)
    inv_all = accpool.tile([P, CT], dtf, tag="inv_all")
    nc.vector.reciprocal(inv_all[:], safe_all[:])
    pres_all = accpool.tile([P, CT], dtf, tag="pres_all")  # 0/1 presence
    nc.vector.tensor_scalar(out=pres_all[:], in0=cnt_all[:], scalar1=0.5,
                            scalar2=None, op0=alu.is_ge)
    ainv_all = accpool.tile([P, CT], dtf, tag="ainv_all")  # alpha*present/cnt
    nc.vector.tensor_tensor(out=ainv_all[:], in0=pres_all[:], in1=inv_all[:],
                            op=alu.mult)
    nc.vector.tensor_scalar_mul(ainv_all[:], ainv_all[:], ALPHA)
    oma_all = accpool.tile([P, CT], dtf, tag="oma_all")    # 1 - alpha*present
    nc.vector.tensor_scalar(out=oma_all[:], in0=pres_all[:], scalar1=-ALPHA,
                            scalar2=1.0, op0=alu.mult, op1=alu.add)

    # loss staging columns
    acc = accpool.tile([P, 4], dtf, tag="acc")
    nc.vector.memset(acc[:], 0.0)
    hsq_stage = accpool.tile([P, nbt], dtf, tag="hsq_stage")
    cross_stage = accpool.tile([P, CT], dtf, tag="cross_stage")
    csq_all = accpool.tile([P, CT], dtf, tag="csq_all")

    # center groups of C_GROUP class tiles; h groups per _h_groups
    hoff = 0  # flat offset into h_blk
    hgi = iter(h_groups)
    ht = None
    ht_cols = ht_base = 0
    pend = []

    assert CT % C_GROUP == 0
    for cg in range(CT // C_GROUP):
        tiles = range(cg * C_GROUP, (cg + 1) * C_GROUP)
        cin = clspool.tile([P, C_GROUP * F], dtf, tag="cin")
        nc.sync.dma_start(
            cin[:], center_in[:, cg * C_GROUP * F:(cg + 1) * C_GROUP * F])
        cout = clspool.tile([P, C_GROUP * F], dtf, tag="cout")

        for j in tiles:
            nj = n_j[j]
            col = col_of[j]
            if ht is None or col >= ht_base + ht_cols:
                g = next(hgi)
                cols = int(sum(n_j[jj] for jj in g))
                ht = hpool.tile([P, cols * F], dtf, tag="ht")
                nc.sync.dma_start(
                    ht[:],
                    h_blk[hoff:hoff + P * cols * F].rearrange(
                        "(p x) -> p x", p=P))
                hoff += P * cols * F
                ht_base, ht_cols = col, cols

            ps = pspool.tile([P, F], dtf, tag="ps")
            for s in range(nj):
                lc = col - ht_base + s
                hsl = ht[:, lc * F:(lc + 1) * F]
                oh = ohpool.tile([P, P], dtf, tag="oh")
                nc.vector.tensor_scalar(
                    out=oh[:], in0=iota_f[:], scalar1=d_all[:, col + s:col + s + 1],
                    scalar2=float(-(j * P)), op0=alu.subtract, op1=alu.is_equal,
                )
                nc.tensor.matmul(ps[:], lhsT=oh[:], rhs=hsl,
                                 start=(s == 0), stop=(s == nj - 1))
                # sum_f h^2, split between ACT (fused accum) and DVE
                if (col + s) % ACT_HSQ_MOD == 0:
                    hsq = ohpool.tile([P, F], dtf, tag="hsq")
                    nc.scalar.activation(
                        hsq[:], hsl, mybir.ActivationFunctionType.Square,
                        accum_out=hsq_stage[:, col + s:col + s + 1])
                else:
                    sq = ohpool.tile([P, F], dtf, tag="sq")
                    nc.vector.tensor_mul(sq[:], hsl, hsl)
                    nc.vector.tensor_reduce(
                        hsq_stage[:, col + s:col + s + 1], sq[:],
                        axis=mybir.AxisListType.X, op=alu.add)

            t = j - cg * C_GROUP
            ctsl = cin[:, t * F:(t + 1) * F]
            # new_c = ainv*sums + oma*c
            t1 = clspool.tile([P, F], dtf, tag="t1")
            nc.vector.tensor_scalar_mul(t1[:], ps[:], ainv_all[:, j:j + 1])
            t2 = clspool.tile([P, F], dtf, tag="t2")
            nc.gpsimd.tensor_scalar_mul(t2[:], ctsl, oma_all[:, j:j + 1])
            nc.gpsimd.tensor_add(cout[:, t * F:(t + 1) * F], t1[:], t2[:])
            # loss: cross_stage[:,j] = sum_f s*c (DVE);
            #       csq_all[:,j] = sum_f c^2 (ACT fused)
            cm = clspool.tile([P, F], dtf, tag="cm")
            nc.vector.tensor_mul(cm[:], ps[:], ctsl)
            nc.vector.tensor_reduce(cross_stage[:, j:j + 1], cm[:],
                                    axis=mybir.AxisListType.X, op=alu.add)
            csq = clspool.tile([P, F], dtf, tag="csq")
            nc.scalar.activation(csq[:], ctsl,
                                 mybir.ActivationFunctionType.Square,
                                 accum_out=csq_all[:, j:j + 1])

        nc.sync.dma_start(
            center_out[:, cg * C_GROUP * F:(cg + 1) * C_GROUP * F], cout[:])

    # fold staging: acc[:,0]=sum h^2, acc[:,1]=sum s.c, acc[:,2]=sum n|c|^2
    nc.vector.tensor_reduce(acc[:, 0:1], hsq_stage[:],
                            axis=mybir.AxisListType.X, op=alu.add)
    nc.vector.tensor_reduce(acc[:, 1:2], cross_stage[:],
                            axis=mybir.AxisListType.X, op=alu.add)
    wc_all = accpool.tile([P, CT], dtf, tag="wc_all")
    nc.vector.tensor_tensor(out=wc_all[:], in0=csq_all[:], in1=cnt_all[:],
                            op=alu.mult)
    nc.vector.tensor_reduce(acc[:, 2:3], wc_all[:], axis=mybir.AxisListType.X,
                            op=alu.add)

    # reduce partition dim: lossp[1,4] = ones.T @ acc
    psl = pslpool.tile([1, 4], dtf, tag="psl")
    nc.tensor.matmul(psl[:], lhsT=ones[:], rhs=acc[:], start=True, stop=True)
    lt = accpool.tile([1, 4], dtf, tag="lt")
    nc.vector.tensor_copy(lt[:], psl[:])
    nc.sync.dma_start(lossp[:], lt[:])


def build_program(n_j, reps=1):
    """Compile the SPMD program for the given static per-class-tile batch
    sub-tile counts. reps>1 wraps the body in a hardware loop (timing)."""
    key = (tuple(n_j), reps)
    if key in _prog_cache:
        return _prog_cache[key]
    import concourse.bacc as bacc
    import concourse.mybir as mybir
    import concourse.tile as tile

    nbt = int(sum(n_j))
    dtf = mybir.dt.float32
    nc = bacc.Bacc("TRN2", target_bir_lowering=False, debug=False,
                   num_devices=N_CORES)
    h_blk = nc.dram_tensor("h_blk", [P * nbt * F], dtf,
                           kind="ExternalInput").ap()
    d_all = nc.dram_tensor("d_all", [P, nbt], dtf, kind="ExternalInput").ap()
    counts = nc.dram_tensor("counts", [P, CT], dtf, kind="ExternalInput").ap()
    center_in = nc.dram_tensor("center_in", [P, CT * F], dtf,
                               kind="ExternalInput").ap()
    center_out = nc.dram_tensor("center_out", [P, CT * F], dtf,
                                kind="ExternalOutput").ap()
    lossp = nc.dram_tensor("lossp", [1, 4], dtf, kind="ExternalOutput").ap()
    aps = (h_blk, d_all, counts, center_in, center_out, lossp)

    with tile.TileContext(nc) as tc:
        with (
            tc.tile_pool(name="const", bufs=1) as cpool,
            tc.tile_pool(name="hp", bufs=3) as hpool,
            tc.tile_pool(name="ohp", bufs=4) as ohpool,
            tc.tile_pool(name="cls", bufs=3) as clspool,
            tc.tile_pool(name="accp", bufs=1) as accpool,
            tc.tile_pool(name="ps", bufs=6, space="PSUM") as pspool,
            tc.tile_pool(name="psl", bufs=1, space="PSUM") as pslpool,
        ):
            pools = (cpool, hpool, ohpool, clspool, accpool, pspool, pslpool)
            if reps == 1:
                _emit_body(nc, tc, mybir, pools, n_j, aps)
            else:
                with tc.For_i(0, reps, 1):
                    _emit_body(nc, tc, mybir, pools, n_j, aps)
    nc.compile()
    _prog_cache[key] = nc
    return nc


def _unshard(results):
    parts = []
    for c in range(N_CORES):
        pm = results[c]["center_out"].reshape(P, CT, F).transpose(1, 0, 2)
        parts.append(pm.reshape(C_PAD, F)[:C_SHARD])
    new_center = np.concatenate(parts, axis=0)
    tot = np.zeros(3, np.float64)
    for c in range(N_CORES):
        lp = results[c]["lossp"][0]
        tot += lp[:3].astype(np.float64)
    loss = (tot[0] - 2.0 * tot[1] + tot[2]) / (BATCH * F)
    return np.float32(loss), new_center


def kernel(h, d, center):
    from concourse.bass_utils import run_bass_kernel_spmd

    in_maps, n_j = _shard_inputs(h, d, center)
    nc = build_program(n_j, reps=1)
    res = run_bass_kernel_spmd(nc, in_maps, core_ids=list(range(N_CORES)))
    return _unshard(res.results)
